# revision 7
# baseline (speedup 1.0000x reference)
"""MDN-RNN mixture-density loss kernel for Trainium2, SPMD over 8 NeuronCores.

Math (per token row i):
    w_d    = exp(-2 * logstd_d)                      [per component k]
    q_d    = (target_d - mean_d)^2
    h_k    = sum_d q_d * w_d     (== sum_d z^2 with z = (t-m)*exp(-logstd))
    sls_k  = sum_d logstd_d
    loss   = -mean_i logsumexp_k(log_mix_coeffs - 0.5*h_k - sls_k)

Sharding: data-parallel on the token dim N=16384 -> 2048 rows per core,
no cross-device communication; each core emits a [128,1] partial sum of
per-row -logsumexp values, combined into the scalar mean on the host.

Schedule: the kernel is HBM-bound (98 MB/core mandatory reads at ~405
GB/s -> ~243 us stream).  Everything is organized to keep the DMA stream
continuous and the compute tail after the last chunk short:

  - DMA per tile is interleaved per mixture component k:
    tgt, mean_0, lstd_0, mean_1, lstd_1, ... so each k's compute chain
    starts as soon as its two chunks land (~1.4 us granularity).
  - Engine split per [128, D] chunk, all under the 15.1 us/tile DMA
    envelope so no engine back-pressures the stream:
      GPSIMD: diff_k = tgt - mean_k       (2-input sub, in place)
      ACT:    w_k = exp(-2*lstd_k); q_k = diff_k^2 (Square, in place)
      DVE:    sls_k = sum(lstd_k); h_k = sum(q_k*w_k) via fused
              tensor_tensor_reduce (accumulator written directly,
              no ACTIVATION_READ_ACCUMULATOR drain)
  - Logsumexp smalls per tile (DVE + one ACT exp w/ accumulate); the Ln
    for log(sum exp) runs once post-loop, and its activation table is
    preloaded via a dummy Ln during tile 0 so the table swap overlaps
    the stream instead of the tail.
"""

import sys

if "/opt/trn_rl_repo" not in sys.path:
    sys.path.insert(0, "/opt/trn_rl_repo")

import numpy as np

N = 16384
K = 5
D = 1088
KD = K * D
NCORES = 8
R = N // NCORES          # 2048 rows per core
P = 128                  # partitions
T = R // P               # 16 tiles per core

import os
SUB_ENGINE = os.environ.get("MDN_SUB_ENGINE", "gpsimd")   # gpsimd | vector
# tensor_tensor_reduce passes CoreSim but dies on this hardware runtime
# (redacted INTERNAL error) — default to the z-form with ACT square-accum
H_VIA_TTR = os.environ.get("MDN_H_VIA_TTR", "0") == "1"

_NC = None


def _build():
    import concourse.bacc as bacc
    import concourse.bass as bass
    import concourse.tile as tile
    from concourse import mybir

    AF = mybir.ActivationFunctionType
    AL = mybir.AluOpType
    AX = mybir.AxisListType
    f32 = mybir.dt.float32

    nc = bacc.Bacc("TRN2", debug=False)
    tgt = nc.dram_tensor("tgt", [R, D], f32, kind="ExternalInput").ap()
    mean = nc.dram_tensor("mean", [R, KD], f32, kind="ExternalInput").ap()
    lstd = nc.dram_tensor("lstd", [R, KD], f32, kind="ExternalInput").ap()
    lmx = nc.dram_tensor("lmx", [P, T * K], f32, kind="ExternalInput").ap()
    out = nc.dram_tensor("partial", [P, 1], f32, kind="ExternalOutput").ap()

    with tile.TileContext(nc) as tc:
        with (
            tc.tile_pool(name="tgt_p", bufs=3) as tgt_p,
            tc.tile_pool(name="mean_p", bufs=3) as mean_p,
            tc.tile_pool(name="lstd_p", bufs=3) as lstd_p,
            tc.tile_pool(name="w_p", bufs=2) as w_p,
            tc.tile_pool(name="small_p", bufs=3) as small_p,
            tc.tile_pool(name="persist", bufs=1) as persist,
        ):
            t_lmx = persist.tile([P, T * K], f32)
            t_nmacc = persist.tile([P, T], f32)   # per-tile -max_k score
            t_sacc = persist.tile([P, T], f32)    # per-tile sum_k exp(score+nm)
            t_scr = persist.tile([P, 1], f32)     # dummy-Ln scratch

            def emit(t):
                rows = slice(t * P, (t + 1) * P)
                t_tgt = tgt_p.tile([P, D], f32)
                t_mean = mean_p.tile([P, K, D], f32)
                t_lstd = lstd_p.tile([P, K, D], f32)
                t_w = w_p.tile([P, K, D], f32)
                t_sls = small_p.tile([P, K], f32)
                t_h = small_p.tile([P, K], f32)
                mean3 = mean[rows].rearrange("p (k d) -> p k d", k=K)
                lstd3 = lstd[rows].rearrange("p (k d) -> p k d", k=K)

                nc.sync.dma_start(out=t_tgt, in_=tgt[rows])
                for k in range(K):
                    nc.sync.dma_start(out=t_mean[:, k, :], in_=mean3[:, k, :])
                    nc.sync.dma_start(out=t_lstd[:, k, :], in_=lstd3[:, k, :])
                    if t == 0 and k == 0:
                        # after the first chunks so it doesn't delay the
                        # stream start; only needed by tile 0's smalls
                        nc.sync.dma_start(out=t_lmx, in_=lmx)

                    # diff_k = tgt - mean_k, in place (GPSIMD)
                    sub_eng = nc.gpsimd if SUB_ENGINE == "gpsimd" else nc.vector
                    sub_eng.tensor_tensor(
                        out=t_mean[:, k, :], in0=t_tgt, in1=t_mean[:, k, :],
                        op=AL.subtract,
                    )
                    # w_k = exp(-2*lstd_k) for the ttr path, exp(-lstd_k) for
                    # the z-form path (ACT)
                    nc.scalar.activation(
                        out=t_w[:, k, :], in_=t_lstd[:, k, :], func=AF.Exp,
                        scale=-2.0 if H_VIA_TTR else -1.0,
                    )
                    # sls_k = sum_d lstd_k (DVE)
                    nc.vector.tensor_reduce(
                        out=t_sls[:, k : k + 1], in_=t_lstd[:, k, :],
                        axis=AX.X, op=AL.add,
                    )
                    if H_VIA_TTR:
                        # q_k = diff_k^2, in place (ACT)
                        nc.scalar.activation(
                            out=t_mean[:, k, :], in_=t_mean[:, k, :], func=AF.Square,
                        )
                        # h_k = sum_d q_k * w_k (DVE fused multiply-reduce)
                        nc.vector.tensor_tensor_reduce(
                            out=t_mean[:, k, :], in0=t_mean[:, k, :],
                            in1=t_w[:, k, :], scale=1.0, scalar=0.0,
                            op0=AL.mult, op1=AL.add,
                            accum_out=t_h[:, k : k + 1],
                        )
                    else:
                        # z_k = diff_k * w_k (DVE), h_k = sum z^2 (ACT accum);
                        # w must then hold exp(-logstd), handled at the exp above
                        nc.vector.tensor_tensor(
                            out=t_mean[:, k, :], in0=t_mean[:, k, :],
                            in1=t_w[:, k, :], op=AL.mult,
                        )
                        nc.scalar.activation(
                            out=t_mean[:, k, :], in_=t_mean[:, k, :], func=AF.Square,
                            accum_out=t_h[:, k : k + 1],
                        )

                # score_k = -0.5*h_k - sls_k + lmx_k ; nm = -max_k score
                t_q = small_p.tile([P, K], f32)
                nc.vector.scalar_tensor_tensor(
                    out=t_q, in0=t_h, scalar=-0.5, in1=t_sls,
                    op0=AL.mult, op1=AL.subtract,
                )
                t_c = small_p.tile([P, K], f32)
                nc.vector.tensor_tensor(
                    out=t_c, in0=t_q, in1=t_lmx[:, t * K : (t + 1) * K], op=AL.add
                )
                nc.vector.tensor_reduce(
                    out=t_nmacc[:, t : t + 1], in_=t_c, axis=AX.X, op=AL.max,
                    negate=True,
                )
                # S_t = sum_k exp(score + nm)
                t_e = small_p.tile([P, K], f32)
                nc.scalar.activation(
                    out=t_e, in_=t_c, func=AF.Exp, bias=t_nmacc[:, t : t + 1],
                    scale=1.0, accum_out=t_sacc[:, t : t + 1],
                )
                if t == 0:
                    # preload the Ln activation table during the stream so
                    # the post-loop Ln doesn't pay the table swap in the tail
                    nc.scalar.activation(out=t_scr, in_=t_nmacc[:, 0:1], func=AF.Ln)

            for t in range(T):
                emit(t)

            # loss rows: nm_t - ln(S_t)
            t_lns = persist.tile([P, T], f32)
            nc.scalar.activation(out=t_lns, in_=t_sacc, func=AF.Ln)
            t_accv = persist.tile([P, T], f32)
            nc.vector.tensor_tensor(out=t_accv, in0=t_nmacc, in1=t_lns, op=AL.subtract)
            t_tot = persist.tile([P, 1], f32)
            nc.vector.tensor_reduce(out=t_tot, in_=t_accv, axis=AX.X, op=AL.add)
            nc.sync.dma_start(out=out, in_=t_tot)

    nc.compile()
    return nc


def get_nc():
    global _NC
    if _NC is None:
        _NC = _build()
    return _NC


def make_in_maps(target, s_mean, s_logstd, log_mix_coeffs):
    target = np.ascontiguousarray(np.asarray(target, dtype=np.float32))
    s_mean = np.ascontiguousarray(np.asarray(s_mean, dtype=np.float32))
    s_logstd = np.ascontiguousarray(np.asarray(s_logstd, dtype=np.float32))
    lm = np.ascontiguousarray(np.asarray(log_mix_coeffs, dtype=np.float32))
    in_maps = []
    for c in range(NCORES):
        rows = slice(c * R, (c + 1) * R)
        # pack log-mix so tile t's [128, K] block sits at columns [t*K, (t+1)*K)
        lmx = lm[rows].reshape(T, P, K).transpose(1, 0, 2).reshape(P, T * K)
        in_maps.append({
            "tgt": np.ascontiguousarray(target[rows]),
            "mean": np.ascontiguousarray(s_mean[rows]),
            "lstd": np.ascontiguousarray(s_logstd[rows]),
            "lmx": np.ascontiguousarray(lmx),
        })
    return in_maps


def combine(results):
    total = sum(float(np.asarray(r["partial"], dtype=np.float64).sum()) for r in results)
    return np.float32(total / N)


def kernel(target, s_mean, s_logstd, log_mix_coeffs):
    from concourse.bass_utils import run_bass_kernel_spmd

    nc = get_nc()
    in_maps = make_in_maps(target, s_mean, s_logstd, log_mix_coeffs)
    res = run_bass_kernel_spmd(nc, in_maps, core_ids=list(range(NCORES)))
    return combine(res.results)


# revision 8
# speedup vs baseline: 1.1130x; 1.1130x over previous
"""MDN-RNN mixture-density loss kernel for Trainium2, SPMD over 8 NeuronCores.

Math (per token row i):
    e1_d   = exp(-logstd_d)            [per component k, bf16]
    z_d    = (target_d - mean_d) * e1_d                 [bf16]
    h_k    = sum_d z_d^2
    sls_k  = sum_d logstd_d
    loss   = -mean_i logsumexp_k(log_mix_coeffs - 0.5*h_k - sls_k)

Sharding: data-parallel on the token dim N=16384 -> 2048 rows per core,
no cross-device communication; each core emits a [128,1] partial sum of
per-row -logsumexp values, combined into the scalar mean on the host.

Schedule: the kernel is HBM-bound (98 MB/core mandatory reads at ~405
GB/s -> ~243 us stream).  Per 128-row tile the DMA is split into seven
chunks (tgt, mean01, lstd01, mean23, lstd23, mean4, lstd4) so each
component's compute chain starts as soon as its chunks land, keeping the
per-tile chain ~6 us past its last chunk - short tail, no special-cased
first/last tile.

Engine split per tile (all under the ~15.1 us/tile DMA envelope so no
engine back-pressures the stream):
    GPSIMD: diff_k = tgt - mean_k   (fp32 in, bf16 out)   ~13.6 us
    ACT:    e1 = exp(-lstd) grouped per chunk (bf16 out),
            h_k = sum z^2 for k=0..3 via Square w/ accumulate ~12.4 us
    DVE:    sls = sum(lstd) grouped per chunk; z_k = diff_k * e1_k
            (bf16, 2x mode); h_4 via bf16 square-mult + reduce;
            logsumexp smalls                               ~12 us

bf16 intermediates: per-element ~0.4% rounding is random-sign across the
1088-wide sums and 16k rows; the end-to-end loss error stays ~1e-5,
far inside the 2e-2 gate, while halving DVE cycles and SBUF port traffic.
"""

import sys

if "/opt/trn_rl_repo" not in sys.path:
    sys.path.insert(0, "/opt/trn_rl_repo")

import numpy as np

N = 16384
K = 5
D = 1088
KD = K * D
NCORES = 8
R = N // NCORES          # 2048 rows per core
P = 128                  # partitions
T = R // P               # 16 tiles per core

# k-chunks: [0,1], [2,3], [4] — DMA/exp/sls granularity
KCH = ((0, 2), (2, 4), (4, 5))

_NC = None


def _build():
    import concourse.bacc as bacc
    import concourse.bass as bass
    import concourse.tile as tile
    from concourse import mybir

    AF = mybir.ActivationFunctionType
    AL = mybir.AluOpType
    AX = mybir.AxisListType
    f32 = mybir.dt.float32
    bf16 = mybir.dt.bfloat16

    nc = bacc.Bacc("TRN2", debug=False)
    tgt = nc.dram_tensor("tgt", [R, D], f32, kind="ExternalInput").ap()
    mean = nc.dram_tensor("mean", [R, KD], f32, kind="ExternalInput").ap()
    lstd = nc.dram_tensor("lstd", [R, KD], f32, kind="ExternalInput").ap()
    lmx = nc.dram_tensor("lmx", [P, T * K], f32, kind="ExternalInput").ap()
    out = nc.dram_tensor("partial", [P, 1], f32, kind="ExternalOutput").ap()

    with tile.TileContext(nc) as tc:
        with (
            tc.tile_pool(name="tgt_p", bufs=3) as tgt_p,
            tc.tile_pool(name="mean_p", bufs=3) as mean_p,
            tc.tile_pool(name="lstd_p", bufs=3) as lstd_p,
            tc.tile_pool(name="z_p", bufs=2) as z_p,
            tc.tile_pool(name="w_p", bufs=2) as w_p,
            tc.tile_pool(name="small_p", bufs=3) as small_p,
            tc.tile_pool(name="persist", bufs=1) as persist,
        ):
            t_lmx = persist.tile([P, T * K], f32)
            t_nmacc = persist.tile([P, T], f32)   # per-tile -max_k score
            t_sacc = persist.tile([P, T], f32)    # per-tile sum_k exp(score+nm)

            def emit(t):
                rows = slice(t * P, (t + 1) * P)
                t_tgt = tgt_p.tile([P, D], f32)
                t_mean = mean_p.tile([P, K, D], f32)
                t_lstd = lstd_p.tile([P, K, D], f32)
                t_z = z_p.tile([P, K, D], bf16)
                t_w = w_p.tile([P, K, D], bf16)
                t_sls = small_p.tile([P, K], f32)
                t_h = small_p.tile([P, K], f32)
                mean3 = mean[rows].rearrange("p (k d) -> p k d", k=K)
                lstd3 = lstd[rows].rearrange("p (k d) -> p k d", k=K)

                # interleaved chunked loads: each k-chunk's chain starts as
                # soon as its mean/lstd pieces land
                nc.sync.dma_start(out=t_tgt, in_=tgt[rows])
                for a, b in KCH:
                    nc.sync.dma_start(out=t_mean[:, a:b, :], in_=mean3[:, a:b, :])
                    nc.sync.dma_start(out=t_lstd[:, a:b, :], in_=lstd3[:, a:b, :])
                    if t == 0 and a == 0:
                        # after the first chunks so it doesn't delay the
                        # stream start; only needed by tile 0's smalls
                        nc.sync.dma_start(out=t_lmx, in_=lmx)

                # GPSIMD: diff_k = tgt - mean_k, bf16 out
                for k in range(K):
                    nc.gpsimd.tensor_tensor(
                        out=t_z[:, k, :], in0=t_tgt, in1=t_mean[:, k, :],
                        op=AL.subtract,
                    )
                # ACT: e1 = exp(-lstd) per chunk, bf16 out — interleaved with
                # the squares below via emission order (see queue comments)
                # DVE: sls per chunk; z_k = diff_k * e1_k (bf16 2x mode)
                # ACT queue: exp01, sq0, exp23, sq1, exp4, sq2, sq3, small
                # DVE queue: sls01, m0, m1, sls23, m2, m3, sls4, m4, sq4(2 ops), smalls
                nc.scalar.activation(
                    out=t_w[:, 0:2, :], in_=t_lstd[:, 0:2, :], func=AF.Exp,
                    scale=-1.0,
                )
                nc.vector.tensor_reduce(
                    out=t_sls[:, 0:2], in_=t_lstd[:, 0:2, :], axis=AX.X, op=AL.add,
                )
                nc.vector.tensor_tensor(
                    out=t_z[:, 0, :], in0=t_z[:, 0, :], in1=t_w[:, 0, :], op=AL.mult,
                )
                nc.scalar.activation(
                    out=t_z[:, 0, :], in_=t_z[:, 0, :], func=AF.Square,
                    accum_out=t_h[:, 0:1],
                )
                nc.vector.tensor_tensor(
                    out=t_z[:, 1, :], in0=t_z[:, 1, :], in1=t_w[:, 1, :], op=AL.mult,
                )
                nc.scalar.activation(
                    out=t_w[:, 2:4, :], in_=t_lstd[:, 2:4, :], func=AF.Exp,
                    scale=-1.0,
                )
                nc.scalar.activation(
                    out=t_z[:, 1, :], in_=t_z[:, 1, :], func=AF.Square,
                    accum_out=t_h[:, 1:2],
                )
                nc.vector.tensor_reduce(
                    out=t_sls[:, 2:4], in_=t_lstd[:, 2:4, :], axis=AX.X, op=AL.add,
                )
                nc.vector.tensor_tensor(
                    out=t_z[:, 2, :], in0=t_z[:, 2, :], in1=t_w[:, 2, :], op=AL.mult,
                )
                nc.scalar.activation(
                    out=t_w[:, 4, :], in_=t_lstd[:, 4, :], func=AF.Exp, scale=-1.0,
                )
                nc.scalar.activation(
                    out=t_z[:, 2, :], in_=t_z[:, 2, :], func=AF.Square,
                    accum_out=t_h[:, 2:3],
                )
                nc.vector.tensor_tensor(
                    out=t_z[:, 3, :], in0=t_z[:, 3, :], in1=t_w[:, 3, :], op=AL.mult,
                )
                nc.vector.tensor_reduce(
                    out=t_sls[:, 4:5], in_=t_lstd[:, 4, :], axis=AX.X, op=AL.add,
                )
                nc.scalar.activation(
                    out=t_z[:, 3, :], in_=t_z[:, 3, :], func=AF.Square,
                    accum_out=t_h[:, 3:4],
                )
                nc.vector.tensor_tensor(
                    out=t_z[:, 4, :], in0=t_z[:, 4, :], in1=t_w[:, 4, :], op=AL.mult,
                )
                # h_4 on DVE (bf16 square then reduce) to balance ACT
                nc.vector.tensor_tensor(
                    out=t_z[:, 4, :], in0=t_z[:, 4, :], in1=t_z[:, 4, :], op=AL.mult,
                )
                nc.vector.tensor_reduce(
                    out=t_h[:, 4:5], in_=t_z[:, 4, :], axis=AX.X, op=AL.add,
                )

                # score_k = -0.5*h_k - sls_k + lmx_k ; nm = -max_k score
                t_q = small_p.tile([P, K], f32)
                nc.vector.scalar_tensor_tensor(
                    out=t_q, in0=t_h, scalar=-0.5, in1=t_sls,
                    op0=AL.mult, op1=AL.subtract,
                )
                t_c = small_p.tile([P, K], f32)
                nc.vector.tensor_tensor(
                    out=t_c, in0=t_q, in1=t_lmx[:, t * K : (t + 1) * K], op=AL.add
                )
                nc.vector.tensor_reduce(
                    out=t_nmacc[:, t : t + 1], in_=t_c, axis=AX.X, op=AL.max,
                    negate=True,
                )
                # S_t = sum_k exp(score + nm)
                t_e = small_p.tile([P, K], f32)
                nc.scalar.activation(
                    out=t_e, in_=t_c, func=AF.Exp, bias=t_nmacc[:, t : t + 1],
                    scale=1.0, accum_out=t_sacc[:, t : t + 1],
                )

            for t in range(T):
                emit(t)

            # loss rows: nm_t - ln(S_t)
            t_lns = persist.tile([P, T], f32)
            nc.scalar.activation(out=t_lns, in_=t_sacc, func=AF.Ln)
            t_accv = persist.tile([P, T], f32)
            nc.vector.tensor_tensor(out=t_accv, in0=t_nmacc, in1=t_lns, op=AL.subtract)
            t_tot = persist.tile([P, 1], f32)
            nc.vector.tensor_reduce(out=t_tot, in_=t_accv, axis=AX.X, op=AL.add)
            nc.sync.dma_start(out=out, in_=t_tot)

    nc.compile()
    return nc


def get_nc():
    global _NC
    if _NC is None:
        _NC = _build()
    return _NC


def make_in_maps(target, s_mean, s_logstd, log_mix_coeffs):
    target = np.ascontiguousarray(np.asarray(target, dtype=np.float32))
    s_mean = np.ascontiguousarray(np.asarray(s_mean, dtype=np.float32))
    s_logstd = np.ascontiguousarray(np.asarray(s_logstd, dtype=np.float32))
    lm = np.ascontiguousarray(np.asarray(log_mix_coeffs, dtype=np.float32))
    in_maps = []
    for c in range(NCORES):
        rows = slice(c * R, (c + 1) * R)
        # pack log-mix so tile t's [128, K] block sits at columns [t*K, (t+1)*K)
        lmx = lm[rows].reshape(T, P, K).transpose(1, 0, 2).reshape(P, T * K)
        in_maps.append({
            "tgt": np.ascontiguousarray(target[rows]),
            "mean": np.ascontiguousarray(s_mean[rows]),
            "lstd": np.ascontiguousarray(s_logstd[rows]),
            "lmx": np.ascontiguousarray(lmx),
        })
    return in_maps


def combine(results):
    total = sum(float(np.asarray(r["partial"], dtype=np.float64).sum()) for r in results)
    return np.float32(total / N)


def kernel(target, s_mean, s_logstd, log_mix_coeffs):
    from concourse.bass_utils import run_bass_kernel_spmd

    nc = get_nc()
    in_maps = make_in_maps(target, s_mean, s_logstd, log_mix_coeffs)
    res = run_bass_kernel_spmd(nc, in_maps, core_ids=list(range(NCORES)))
    return combine(res.results)


# revision 10
# speedup vs baseline: 1.2872x; 1.1565x over previous
"""MDN-RNN mixture-density loss kernel for Trainium2, SPMD over 8 NeuronCores.

Math (per token row i):
    e1_d   = exp(-logstd_d)            [per component k, bf16]
    z_d    = (target_d - mean_d) * e1_d                 [bf16]
    h_k    = sum_d z_d^2
    sls_k  = sum_d logstd_d
    loss   = -mean_i logsumexp_k(log_mix_coeffs - 0.5*h_k - sls_k)

Sharding: data-parallel on the token dim N=16384 -> 2048 rows per core.
Each core emits nm = -max_k(score) and S = sum_k exp(score+nm) per row,
packed [128, 2T]; the host finishes loss = -mean(-nm + ln S) (a 16k-ln
host reduction, same category as the existing partial-sum combine).

Schedule notes (HBM-bound: 98 MB/core at ~400 GB/s -> ~245-250 us
stream; every engine must stay under the ~15.6 us/tile DMA envelope):

  - DMA per tile in 7 chunks (tgt, lstd01, mean01, lstd23, mean23,
    lstd4, mean4) so per-k chains start as chunks land.
  - GPSIMD 2-input ops share an SBUF port with DVE and throttle DVE
    ~4x when overlapped (hardware-measured), so gpsimd gets only the
    tiny [128,5] logsumexp ops; all heavy elementwise stays on DVE/ACT:
      DVE: sub_k (fp32->bf16), z_k = diff*e1 (bf16 2x mode, 725 ns
           measured), sls01/sls23 grouped reduces         ~15.0 us/tile
      ACT: exp chunks (bf16 out), h_k squares w/ accumulate,
           sls4 via Copy w/ accumulate                    ~14.9 us/tile
  - No Ln on device -> single activation table set, no mid-kernel or
    tail table swaps.

bf16 intermediates: ~0.4% per-element rounding is random-sign across
1088-wide sums and 16k rows; end-to-end loss error ~1e-4, far inside
the 2e-2 gate, while halving DVE mult cycles and SBUF port traffic.
"""

import sys

if "/opt/trn_rl_repo" not in sys.path:
    sys.path.insert(0, "/opt/trn_rl_repo")

import numpy as np

N = 16384
K = 5
D = 1088
KD = K * D
NCORES = 8
R = N // NCORES          # 2048 rows per core
P = 128                  # partitions
T = R // P               # 16 tiles per core

_NC = None


def _build():
    import concourse.bacc as bacc
    import concourse.tile as tile
    from concourse import mybir

    AF = mybir.ActivationFunctionType
    AL = mybir.AluOpType
    AX = mybir.AxisListType
    f32 = mybir.dt.float32
    bf16 = mybir.dt.bfloat16

    nc = bacc.Bacc("TRN2", debug=False)
    tgt = nc.dram_tensor("tgt", [R, D], f32, kind="ExternalInput").ap()
    mean = nc.dram_tensor("mean", [R, KD], f32, kind="ExternalInput").ap()
    lstd = nc.dram_tensor("lstd", [R, KD], f32, kind="ExternalInput").ap()
    lmx = nc.dram_tensor("lmx", [P, T * K], f32, kind="ExternalInput").ap()
    out = nc.dram_tensor("res", [P, 2 * T], f32, kind="ExternalOutput").ap()

    with tile.TileContext(nc) as tc:
        with (
            tc.tile_pool(name="tgt_p", bufs=3) as tgt_p,
            tc.tile_pool(name="mean_p", bufs=3) as mean_p,
            tc.tile_pool(name="lstd_p", bufs=3) as lstd_p,
            tc.tile_pool(name="z_p", bufs=2) as z_p,
            tc.tile_pool(name="w_p", bufs=2) as w_p,
            tc.tile_pool(name="small_p", bufs=3) as small_p,
            tc.tile_pool(name="persist", bufs=1) as persist,
        ):
            t_lmx = persist.tile([P, T * K], f32)
            t_res = persist.tile([P, 2 * T], f32)  # [:, :T]=nm, [:, T:]=sacc

            def emit(t):
                rows = slice(t * P, (t + 1) * P)
                t_tgt = tgt_p.tile([P, D], f32)
                t_mean = mean_p.tile([P, K, D], f32)
                t_lstd = lstd_p.tile([P, K, D], f32)
                t_z = z_p.tile([P, K, D], bf16)
                t_w = w_p.tile([P, K, D], bf16)
                t_sls = small_p.tile([P, K], f32)
                t_h = small_p.tile([P, K], f32)
                mean3 = mean[rows].rearrange("p (k d) -> p k d", k=K)
                lstd3 = lstd[rows].rearrange("p (k d) -> p k d", k=K)

                # chunked loads: lstd chunks first (feed exp + sls), then
                # mean chunks (feed the sub/mult/square chain)
                nc.sync.dma_start(out=t_tgt, in_=tgt[rows])
                nc.sync.dma_start(out=t_lstd[:, 0:2, :], in_=lstd3[:, 0:2, :])
                nc.sync.dma_start(out=t_mean[:, 0:2, :], in_=mean3[:, 0:2, :])
                if t == 0:
                    # after the first chunks so it doesn't delay the stream
                    # start; only needed by tile 0's logsumexp smalls
                    nc.sync.dma_start(out=t_lmx, in_=lmx)
                nc.sync.dma_start(out=t_lstd[:, 2:4, :], in_=lstd3[:, 2:4, :])
                nc.sync.dma_start(out=t_mean[:, 2:4, :], in_=mean3[:, 2:4, :])
                nc.sync.dma_start(out=t_lstd[:, 4, :], in_=lstd3[:, 4, :])
                nc.sync.dma_start(out=t_mean[:, 4, :], in_=mean3[:, 4, :])

                def sub(k):
                    nc.vector.tensor_tensor(
                        out=t_z[:, k, :], in0=t_tgt, in1=t_mean[:, k, :],
                        op=AL.subtract,
                    )

                def mul(k):
                    nc.vector.tensor_tensor(
                        out=t_z[:, k, :], in0=t_z[:, k, :], in1=t_w[:, k, :],
                        op=AL.mult,
                    )

                def exp(a, b):
                    nc.scalar.activation(
                        out=t_w[:, a:b, :], in_=t_lstd[:, a:b, :], func=AF.Exp,
                        scale=-1.0,
                    )

                def sq(k):
                    nc.scalar.activation(
                        out=t_z[:, k, :], in_=t_z[:, k, :], func=AF.Square,
                        accum_out=t_h[:, k : k + 1],
                    )

                def sls(a, b):
                    nc.vector.tensor_reduce(
                        out=t_sls[:, a:b], in_=t_lstd[:, a:b, :], axis=AX.X,
                        op=AL.add,
                    )

                # DVE queue: sls01 sub0 m0 sub1 m1 sls23 sub2 m2 sub3 m3 sub4 m4 max
                # ACT queue: exp01 sq0 sq1 exp23 exp4 sq2 sq3 sls4 sq4 small-exp
                # GP queue:  stt add (tiny [128,5] ops; negligible port traffic)
                sls(0, 2)
                exp(0, 2)
                sub(0); mul(0)
                sq(0)
                sub(1); mul(1)
                sq(1)
                sls(2, 4)
                exp(2, 4)
                exp(4, 5)
                sub(2); mul(2)
                sq(2)
                sub(3); mul(3)
                sq(3)
                nc.scalar.activation(
                    out=t_lstd[:, 4, :], in_=t_lstd[:, 4, :], func=AF.Copy,
                    accum_out=t_sls[:, 4:5],
                )
                sub(4); mul(4)
                sq(4)

                # score_k = -0.5*h_k - sls_k + lmx_k ; nm = -max_k score
                t_q = small_p.tile([P, K], f32)
                nc.vector.scalar_tensor_tensor(
                    out=t_q, in0=t_h, scalar=-0.5, in1=t_sls,
                    op0=AL.mult, op1=AL.subtract,
                )
                t_c = small_p.tile([P, K], f32)
                nc.gpsimd.tensor_tensor(
                    out=t_c, in0=t_q, in1=t_lmx[:, t * K : (t + 1) * K], op=AL.add
                )
                nc.vector.tensor_reduce(
                    out=t_res[:, t : t + 1], in_=t_c, axis=AX.X, op=AL.max,
                    negate=True,
                )
                # S_t = sum_k exp(score + nm)
                t_e = small_p.tile([P, K], f32)
                nc.scalar.activation(
                    out=t_e, in_=t_c, func=AF.Exp, bias=t_res[:, t : t + 1],
                    scale=1.0, accum_out=t_res[:, T + t : T + t + 1],
                )

            for t in range(T):
                emit(t)

            nc.sync.dma_start(out=out, in_=t_res)

    nc.compile()
    return nc


def get_nc():
    global _NC
    if _NC is None:
        _NC = _build()
    return _NC


def make_in_maps(target, s_mean, s_logstd, log_mix_coeffs):
    target = np.ascontiguousarray(np.asarray(target, dtype=np.float32))
    s_mean = np.ascontiguousarray(np.asarray(s_mean, dtype=np.float32))
    s_logstd = np.ascontiguousarray(np.asarray(s_logstd, dtype=np.float32))
    lm = np.ascontiguousarray(np.asarray(log_mix_coeffs, dtype=np.float32))
    in_maps = []
    for c in range(NCORES):
        rows = slice(c * R, (c + 1) * R)
        # pack log-mix so tile t's [128, K] block sits at columns [t*K, (t+1)*K)
        lmx = lm[rows].reshape(T, P, K).transpose(1, 0, 2).reshape(P, T * K)
        in_maps.append({
            "tgt": np.ascontiguousarray(target[rows]),
            "mean": np.ascontiguousarray(s_mean[rows]),
            "lstd": np.ascontiguousarray(s_logstd[rows]),
            "lmx": np.ascontiguousarray(lmx),
        })
    return in_maps


def combine(results):
    # res[:, :T] = nm = -max_k score ; res[:, T:] = S = sum_k exp(score+nm)
    # lse = -nm + ln(S); loss = -mean(lse) = mean(nm - ln(S))
    total = 0.0
    for r in results:
        res = np.asarray(r["res"], dtype=np.float64)
        nm, s = res[:, :T], res[:, T:]
        total += float((nm - np.log(s)).sum())
    return np.float32(total / N)


def kernel(target, s_mean, s_logstd, log_mix_coeffs):
    from concourse.bass_utils import run_bass_kernel_spmd

    nc = get_nc()
    in_maps = make_in_maps(target, s_mean, s_logstd, log_mix_coeffs)
    res = run_bass_kernel_spmd(nc, in_maps, core_ids=list(range(NCORES)))
    return combine(res.results)
